# revision 14
# baseline (speedup 1.0000x reference)
"""PointNet++ feature-propagation decoder (4 kNN-interp stages) on 8 trn2 cores.

Data-parallel over batch: B=16 -> 2 batch elements per NeuronCore.
Self-contained: hardcodes all shapes from the problem spec.

Per-core pipeline (per batch element):
  stage i: d = cdist(fine, coarse) via PE matmul of lifted coords;
           top-3 via DVE max8/max_index on PSUM; w = normalized inverse dists;
           gather table rows via indirect DMA (SWDGE); weighted-sum via PE
           matmuls against diag(w_k) accumulated in PSUM.
  Intermediate stage outputs are written row-major [N, C] to a DRAM table
  (the next stage's gather source); the final stage emits channel-major
  [C, N] tiles (lhsT = gathered rows) directly into the output layout.

Schedule: stage 3 of element 0 (DMA-bound: 47MB of gathers + 32MB of output
writes) is interleaved at issue time with the prep work of element 1
(PE-bound: table transposes, coord lifts, stages 0-2), so the in-order
engines overlap the two phases instead of serializing them.
"""

import os
from contextlib import ExitStack

import numpy as np

P = 128
NB = 2          # batch elements per core
NCORES = 8
B = 16

# table dtype for gathers / interp matmuls; float16 halves the dominant
# gather traffic at ~5e-4 relative error (float32 = exact but ~1.6x slower;
# bfloat16 measured the same speed as float16 at 2.8e-3 error)
TBL = os.environ.get("KNN_TBL_DT", "float16")

# stage tuples: (fine xyz idx, coarse xyz idx, N, S, skipC, C2)
#   table_i: [S, C2]; interp_i: [N, C2]; next table = [N, skipC + C2]
STAGES = [
    (3, 4, 64, 16, 512, 1024),
    (2, 3, 256, 64, 256, 1536),
    (1, 2, 1024, 256, 128, 1792),
    (0, 1, 4096, 1024, 64, 1920),
]
XYZ_N = [4096, 1024, 256, 64, 16]
F_C = [64, 128, 256, 512, 1024]
OUT_C = 1984
OUT_N = 4096

_SENT = object()


def build_nc(nb=NB):
    import concourse.bass as bass
    import concourse.bacc as bacc
    import concourse.mybir as mybir
    import concourse.tile as tile
    from concourse.masks import make_identity

    f32 = mybir.dt.float32
    u32 = mybir.dt.uint32
    tdt = getattr(mybir.dt, TBL)
    J = 2 if TBL == "float32" else 4  # n-tiles per gather group

    nc = bacc.Bacc("TRN2")

    xyz_in = [
        nc.dram_tensor(f"xyz{i}", [nb, XYZ_N[i], 3], f32, kind="ExternalInput")
        for i in range(5)
    ]
    f_in = [
        nc.dram_tensor(f"f{i}", [nb, F_C[i], XYZ_N[i]], f32, kind="ExternalInput")
        for i in range(5)
    ]
    out_d = nc.dram_tensor("out", [nb, OUT_C, OUT_N], f32, kind="ExternalOutput")
    # gather tables (stage i gathers from tbl[i]); batch folded into rows so
    # the indirect-DMA offset AP stays the HW-proven 2D [p,1] shape
    # row stride padded to 4096B (2048 fp16) so every gathered row starts
    # page-aligned and a 3840B read never straddles two 4KB pages
    tblpad = [1024, 2048, 2048, 2048]
    tbl = [
        nc.dram_tensor(
            f"tbl{i}", [nb * STAGES[i][3], tblpad[i]], tdt, kind="Internal"
        )
        for i in range(4)
    ]

    AX = mybir.AxisListType.X
    MUL = mybir.AluOpType.mult
    ADD = mybir.AluOpType.add

    with tile.TileContext(nc) as tc, ExitStack() as ctx:
        ident_p = ctx.enter_context(tc.tile_pool(name="ident", bufs=1))
        widep = ctx.enter_context(tc.tile_pool(name="wide", bufs=2))
        abtp = ctx.enter_context(tc.tile_pool(name="abt", bufs=2))
        abtp1 = ctx.enter_context(tc.tile_pool(name="abt1", bufs=2))
        idxp = ctx.enter_context(tc.tile_pool(name="idxp", bufs=2))
        smallp = ctx.enter_context(tc.tile_pool(name="smallp", bufs=4))
        diagp = ctx.enter_context(tc.tile_pool(name="diagp", bufs=3))
        gp = ctx.enter_context(tc.tile_pool(name="gp", bufs=5 * J))
        tcp = ctx.enter_context(tc.tile_pool(name="tcp", bufs=4))
        stagp = ctx.enter_context(tc.tile_pool(name="stagp", bufs=4))
        fblkp = ctx.enter_context(tc.tile_pool(name="fblkp", bufs=2))
        dpool = ctx.enter_context(tc.tile_pool(name="dpool", bufs=2, space="PSUM"))
        mpool = ctx.enter_context(tc.tile_pool(name="mpool", bufs=4, space="PSUM"))

        ident = ident_p.tile([P, P], f32)
        make_identity(nc, ident[:])
        # constant [P, J, 3, P] identity mask used to expand w into diag(w_k)
        idmask = ident_p.tile([P, J, 3, P], tdt)
        for j in range(J):
            for k in range(3):
                nc.vector.tensor_copy(idmask[:, j, k, :], ident[:])

        def pe_transpose_to(dst_ap, src_ap):
            """dst[m, k] = src[k, m] via PE; src [K(partition), M<=128]."""
            k_, m = src_ap.shape[0], src_ap.shape[-1]
            pt = mpool.tile([P, P], f32, tag="po")
            nc.tensor.transpose(pt[:m, :k_], src_ap, ident[:k_, :k_])
            nc.vector.tensor_copy(dst_ap, pt[:m, :k_])

        def pe_transpose_merge(dst_ap, srcs):
            """dst[:, j*k:(j+1)*k] = srcs[j]^T for up to 4 same-shape srcs,
            via one PSUM tile and ONE DVE copy (fewer instructions -> fewer
            semaphore stalls than per-block pe_transpose_to)."""
            k_, m = srcs[0].shape[0], srcs[0].shape[-1]
            pt = mpool.tile([P, 4 * P], f32, tag="po")
            for j, s in enumerate(srcs):
                nc.tensor.transpose(pt[:m, j * k_:(j + 1) * k_], s, ident[:k_, :k_])
            nc.vector.tensor_copy(dst_ap, pt[:m, :len(srcs) * k_])

        def table_chunks(b):
            """Skip-feature table blocks (f^T); yields between blocks."""
            # tbl0 = f4^T entirely: f4 [1024, 16] -> [16, 1024]
            f4w = widep.tile([P, 8, 16], f32, tag="f4w")
            nc.sync.dma_start(f4w[:], f_in[4][b].rearrange("(u p) s -> p u s", p=P))
            t0sb = fblkp.tile([16, 1024], tdt, tag="t0sb")
            for u0 in range(0, 8, 4):
                pe_transpose_merge(
                    t0sb[:, u0 * P:(u0 + 4) * P],
                    [f4w[:, u, :] for u in range(u0, u0 + 4)],
                )
            nc.sync.dma_start(tbl[0][b * 16:(b + 1) * 16, :], t0sb[:])
            yield

            # f3 [512, 64] -> tbl1[:, 0:512]
            f3w = widep.tile([P, 4, 64], f32, tag="f3w")
            nc.sync.dma_start(f3w[:], f_in[3][b].rearrange("(u p) n -> p u n", p=P))
            t1sb = fblkp.tile([64, 512], tdt, tag="t1sb")
            pe_transpose_merge(t1sb[:], [f3w[:, u, :] for u in range(4)])
            nc.sync.dma_start(tbl[1][b * 64:(b + 1) * 64, 0:512], t1sb[:])
            yield

            # f2 [256, 256] -> tbl2[:, 0:256]
            f2w = widep.tile([P, 2, 256], f32, tag="f2w")
            nc.sync.dma_start(f2w[:], f_in[2][b].rearrange("(u p) n -> p u n", p=P))
            for r in range(2):
                t2sb = fblkp.tile([P, 256], tdt, tag="t2sb")
                pe_transpose_merge(
                    t2sb[:], [f2w[:, u, r * P:(r + 1) * P] for u in range(2)])
                nc.sync.dma_start(
                    tbl[2][b * 256 + r * P:b * 256 + (r + 1) * P, 0:256], t2sb[:]
                )
            yield

            # f1 [128, 1024] -> tbl3[:, 0:128]
            f1w = widep.tile([P, 1024], f32, tag="f1w")
            nc.sync.dma_start(f1w[:], f_in[1][b])
            for r0 in range(0, 8, 4):
                t3sb = fblkp.tile([P, 4, P], tdt, tag="t3sb")
                pe_transpose_merge(
                    t3sb[:].rearrange("p j c -> p (j c)"),
                    [f1w[:, r * P:(r + 1) * P] for r in range(r0, r0 + 4)],
                )
                nc.sync.dma_start(
                    tbl[3][
                        b * 1024 + r0 * P:b * 1024 + (r0 + 4) * P, 0:128
                    ].rearrange("(j p) c -> p j c", p=P),
                    t3sb[:],
                )
            yield

            # f0 [64, 4096] -> out rows 0:64 (already channel-major);
            # direct DRAM->DRAM, no SBUF bounce
            for h in range(4):
                nc.sync.dma_start(
                    out_d[b, 0:64, h * 1024:(h + 1) * 1024],
                    f_in[0][b, :, h * 1024:(h + 1) * 1024],
                )
            yield

        def coord_chunks(b, a1t, b2t):
            """Lifted coord operands A'(fine), B'(coarse); yields per level.

            A' = [2x, 2y, 2z, -1, -|p|^2], B' = [x, y, z, |p|^2, 1]
            so that A'.B' = 2ab - |a|^2 - |b|^2 = -d^2
            """
            for i in range(5):
                n = XYZ_N[i]
                t = max(1, n // P)
                pt_ = min(n, P)
                xw = widep.tile([pt_, t, 3], f32, tag="xw")
                nc.sync.dma_start(
                    xw[:], xyz_in[i][b].rearrange("(t p) d -> p t d", p=pt_)
                )
                sq = widep.tile([pt_, t, 3], f32, tag="sq")
                nc.vector.tensor_mul(sq[:], xw[:], xw[:])
                ssum = widep.tile([pt_, t, 1], f32, tag="ssum")
                nc.vector.reduce_sum(ssum[:], sq[:], axis=AX)
                if i != 4:  # used as fine
                    aw = widep.tile([pt_, t, 5], f32, tag="aw")
                    nc.vector.tensor_scalar_mul(aw[:, :, 0:3], xw[:], 2.0)
                    nc.vector.memset(aw[:, :, 3:4], -1.0)
                    nc.vector.tensor_scalar_mul(aw[:, :, 4:5], ssum[:], -1.0)
                    at = (abtp1 if n >= 4096 else abtp).tile(
                        [5, n], f32, tag=f"a1t{i}"
                    )
                    for t0_ in range(0, t, 4):
                        tn = min(4, t - t0_)
                        pe_transpose_merge(
                            at[:, t0_ * pt_:(t0_ + tn) * pt_],
                            [aw[:, tt, :] for tt in range(t0_, t0_ + tn)],
                        )
                    a1t[i] = at
                if i != 0:  # used as coarse
                    bw = widep.tile([pt_, t, 5], f32, tag="bw")
                    nc.vector.tensor_copy(bw[:, :, 0:3], xw[:])
                    nc.vector.tensor_copy(bw[:, :, 3:4], ssum[:])
                    nc.vector.memset(bw[:, :, 4:5], 1.0)
                    bt = (abtp1 if n >= 1024 else abtp).tile(
                        [5, n], f32, tag=f"b2t{i}"
                    )
                    for t0_ in range(0, t, 4):
                        tn = min(4, t - t0_)
                        pe_transpose_merge(
                            bt[:, t0_ * pt_:(t0_ + tn) * pt_],
                            [bw[:, tt, :] for tt in range(t0_, t0_ + tn)],
                        )
                    b2t[i] = bt
                yield

        def make_topk(b, i, at, bt, S, pt_, idxall, valsall):
            def do_topk(t):
                dps = dpool.tile([pt_, S], f32, tag="d")
                for s0 in range(0, S, 512):
                    sw = min(512, S - s0)
                    nc.tensor.matmul(
                        dps[:, s0:s0 + sw],
                        lhsT=at[:, t * pt_:t * pt_ + pt_],
                        rhs=bt[:, s0:s0 + sw],
                        start=True,
                        stop=True,
                    )
                nc.vector.max(out=valsall[:, t, :], in_=dps[:])
                nc.vector.max_index(
                    out=idxall[:, t, :], in_max=valsall[:, t, :], in_values=dps[:]
                )
                if b > 0:
                    # bias to batch-folded table rows; per-tile so gathers
                    # don't wait on the whole stage's topk
                    nc.vector.tensor_scalar_add(
                        idxall[:, t, :], idxall[:, t, :], b * S
                    )
            return do_topk

        def group_weights(valsall, t0, jg, pt_):
            """w = r / sum(r), r = 1/(d + 1e-8), expanded to diag(w_k)."""
            vsl = valsall[:, t0:t0 + jg, :]
            dk = smallp.tile([pt_, jg, 3], f32, tag="dk")
            nc.vector.tensor_scalar(
                out=dk[:], in0=vsl[:, :, 0:3],
                scalar1=-1.0, scalar2=1e-8, op0=MUL, op1=ADD,
            )
            rr = smallp.tile([pt_, jg, 3], f32, tag="rr")
            nc.vector.reciprocal(rr[:], dk[:])
            zz = smallp.tile([pt_, jg, 1], f32, tag="zz")
            nc.vector.reduce_sum(zz[:], rr[:], axis=AX)
            zr = smallp.tile([pt_, jg, 1], f32, tag="zr")
            nc.vector.reciprocal(zr[:], zz[:])
            w3 = smallp.tile([pt_, jg, 3, 1], tdt, tag="w3")
            nc.vector.tensor_tensor(
                out=w3[:, :, :, 0], in0=rr[:],
                in1=zr[:].to_broadcast([pt_, jg, 3]), op=MUL,
            )
            diag = diagp.tile([pt_, jg, 3, pt_], tdt, tag="diag")
            nc.vector.tensor_tensor(
                out=diag[:],
                in0=w3[:].to_broadcast([pt_, jg, 3, pt_]),
                in1=idmask[:pt_, :jg, :, :pt_],
                op=MUL,
            )
            return diag

        def gather_group(i, idxall, t0, jg, pt_, C2):
            # NOTE: the offset AP must stay the HW-proven 2D [p,1] shape; a
            # [p,3] offset AP hard-crashes the exec unit (NRT status 101)
            gs = []
            for tt in range(jg):
                gk = []
                for k in range(3):
                    g = gp.tile([pt_, C2], tdt, tag="g")
                    nc.gpsimd.indirect_dma_start(
                        out=g[:],
                        out_offset=None,
                        in_=tbl[i][:],
                        in_offset=bass.IndirectOffsetOnAxis(
                            ap=idxall[:, t0 + tt, k:k + 1], axis=0
                        ),
                    )
                    gk.append(g)
                gs.append(gk)
            return gs

        def stage_small(b, i, a1t, b2t):
            """Stages 0-2: row-major interp -> next table rows; yields/group."""
            fi, ci, N, S, skipC, C2 = STAGES[i]
            T = max(1, N // P)
            pt_ = min(N, P)
            at, bt = a1t[fi], b2t[ci]

            idxall = idxp.tile([pt_, T, 8], u32, tag=f"idx{i}")
            valsall = smallp.tile([pt_, T, 8], f32, tag=f"vals{i}")
            do_topk = make_topk(b, i, at, bt, S, pt_, idxall, valsall)

            for t in range(min(2 * J, T)):
                do_topk(t)

            for t0 in range(0, T, J):
                jg = min(J, T - t0)
                for t in range(t0 + 2 * J, min(t0 + 3 * J, T)):
                    do_topk(t)
                diag = group_weights(valsall, t0, jg, pt_)
                gs = gather_group(i, idxall, t0, jg, pt_, C2)
                for tt in range(jg):
                    tcst = tcp.tile([pt_, C2], tdt, tag="tc")
                    for c0 in range(0, C2, 512):
                        cw = min(512, C2 - c0)
                        pint = mpool.tile([pt_, 512], f32, tag="po")
                        for k in range(3):
                            nc.tensor.matmul(
                                pint[:, :cw],
                                lhsT=diag[:, tt, k, :],
                                rhs=gs[tt][k][:, c0:c0 + cw],
                                start=(k == 0),
                                stop=(k == 2),
                            )
                        nc.scalar.copy(tcst[:, c0:c0 + cw], pint[:, :cw])
                    r0 = b * STAGES[i + 1][3] + (t0 + tt) * pt_
                    nc.sync.dma_start(
                        tbl[i + 1][r0:r0 + pt_, skipC:skipC + C2], tcst[:]
                    )
                yield

        def stage3(b, a1t, b2t):
            """Final stage: channel-major output rows 64:1984; yields/group.

            topk is software-pipelined two groups ahead of the gathers so PE
            never sits in a stage-wide cdist->topk stall phase.
            """
            fi, ci, N, S, skipC, C2 = STAGES[3]
            T = N // P
            at, bt = a1t[fi], b2t[ci]

            idxall = idxp.tile([P, T, 8], u32, tag="idx3")
            valsall = smallp.tile([P, T, 8], f32, tag="vals3")
            do_topk = make_topk(b, 3, at, bt, S, P, idxall, valsall)

            for t in range(2 * J):
                do_topk(t)
            yield

            for t0 in range(0, T, J):
                jg = min(J, T - t0)
                for t in range(t0 + 2 * J, min(t0 + 3 * J, T)):
                    do_topk(t)
                diag = group_weights(valsall, t0, jg, P)
                gs = gather_group(3, idxall, t0, jg, P, C2)
                for cc in range(C2 // P):
                    pout = mpool.tile([P, 512], f32, tag="po")
                    for tt in range(jg):
                        for k in range(3):
                            nc.tensor.matmul(
                                pout[:, tt * P:(tt + 1) * P],
                                lhsT=gs[tt][k][:, cc * P:(cc + 1) * P],
                                rhs=diag[:, tt, k, :],
                                start=(k == 0),
                                stop=(k == 2),
                            )
                    stg = stagp.tile([P, J * P], f32, tag="stag")
                    cp = nc.scalar.copy if cc % 2 else nc.vector.tensor_copy
                    cp(stg[:, :jg * P], pout[:, :jg * P])
                    nc.sync.dma_start(
                        out_d[
                            b,
                            64 + cc * P:64 + (cc + 1) * P,
                            t0 * P:(t0 + jg) * P,
                        ],
                        stg[:, :jg * P],
                    )
                yield

        def prep(b, a1t, b2t):
            yield from table_chunks(b)
            yield from coord_chunks(b, a1t, b2t)
            for i in range(3):
                yield from stage_small(b, i, a1t, b2t)

        a1t = [{} for _ in range(nb)]
        b2t = [{} for _ in range(nb)]

        for _ in prep(0, a1t[0], b2t[0]):
            pass

        # interleave: stage 3 of elem 0 (DMA-bound) with prep of elem 1
        # (PE-bound), front-loaded 3 prep chunks per stage-3 group so elem 1's
        # stage chain completes early; then bridge elem 1's stage 3 into elem
        # 0's remaining groups so the gather stream never gaps.
        s30 = stage3(0, a1t[0], b2t[0])
        if nb > 1:
            p1 = prep(1, a1t[1], b2t[1])
            while next(p1, _SENT) is not _SENT:
                for _ in range(5):
                    next(p1, None)
                next(s30, None)
            s31 = stage3(1, a1t[1], b2t[1])
            while True:
                a_done = next(s30, _SENT) is _SENT
                b_done = next(s31, _SENT) is _SENT
                if a_done and b_done:
                    break
        else:
            for _ in s30:
                pass

    nc.compile()
    return nc


_CACHE = {}


def _get_nc():
    if "nc" not in _CACHE:
        _CACHE["nc"] = build_nc(NB)
    return _CACHE["nc"]


def run(inputs, trace=False):
    from concourse.bass_utils import run_bass_kernel_spmd

    if not trace:
        # the axon trace path needs an antenv.axon_hooks shim this image may
        # lack; make plain runs immune to a stray BASS_TRACE in the env
        os.environ["BASS_NEVER_TRACE"] = "1"
    else:
        os.environ.pop("BASS_NEVER_TRACE", None)
    nc = _get_nc()
    in_maps = []
    for c in range(NCORES):
        sl = slice(c * NB, (c + 1) * NB)
        m = {}
        for i in range(5):
            m[f"xyz{i}"] = np.ascontiguousarray(inputs[f"xyz{i}"][sl], dtype=np.float32)
            m[f"f{i}"] = np.ascontiguousarray(inputs[f"f{i}"][sl], dtype=np.float32)
        in_maps.append(m)
    res = run_bass_kernel_spmd(nc, in_maps, core_ids=list(range(NCORES)), trace=trace)
    out = np.concatenate([r["out"] for r in res.results], axis=0)
    return out, res


def kernel(**inputs) -> np.ndarray:
    out, _ = run(inputs)
    return out


# revision 16
# speedup vs baseline: 1.0603x; 1.0603x over previous
"""PointNet++ feature-propagation decoder (4 kNN-interp stages) on 8 trn2 cores.

Data-parallel over batch: B=16 -> 2 batch elements per NeuronCore.
Self-contained: hardcodes all shapes from the problem spec.

Per-core pipeline (per batch element):
  stage i: d = cdist(fine, coarse) via PE matmul of lifted coords;
           top-3 via DVE max8/max_index on PSUM; w = normalized inverse dists;
           gather table rows via indirect DMA (SWDGE); weighted-sum via PE
           matmuls against diag(w_k) accumulated in PSUM.
  Intermediate stage outputs are written row-major [N, C] to a DRAM table
  (the next stage's gather source); the final stage emits channel-major
  [C, N] tiles (lhsT = gathered rows) directly into the output layout.

Schedule: stage 3 of element 0 (DMA-bound: 47MB of gathers + 32MB of output
writes) is interleaved at issue time with the prep work of element 1
(PE-bound: table transposes, coord lifts, stages 0-2), so the in-order
engines overlap the two phases instead of serializing them.
"""

import os
from contextlib import ExitStack

import numpy as np

P = 128
NB = 2          # batch elements per core
NCORES = 8
B = 16

# table dtype for gathers / interp matmuls; float16 halves the dominant
# gather traffic at ~5e-4 relative error (float32 = exact but ~1.6x slower;
# bfloat16 measured the same speed as float16 at 2.8e-3 error)
TBL = os.environ.get("KNN_TBL_DT", "float16")

# stage tuples: (fine xyz idx, coarse xyz idx, N, S, skipC, C2)
#   table_i: [S, C2]; interp_i: [N, C2]; next table = [N, skipC + C2]
STAGES = [
    (3, 4, 64, 16, 512, 1024),
    (2, 3, 256, 64, 256, 1536),
    (1, 2, 1024, 256, 128, 1792),
    (0, 1, 4096, 1024, 64, 1920),
]
XYZ_N = [4096, 1024, 256, 64, 16]
F_C = [64, 128, 256, 512, 1024]
OUT_C = 1984
OUT_N = 4096

_SENT = object()


def build_nc(nb=NB):
    import concourse.bass as bass
    import concourse.bacc as bacc
    import concourse.mybir as mybir
    import concourse.tile as tile
    from concourse.masks import make_identity

    f32 = mybir.dt.float32
    u32 = mybir.dt.uint32
    tdt = getattr(mybir.dt, TBL)
    J = 2 if TBL == "float32" else 4  # n-tiles per gather group

    nc = bacc.Bacc("TRN2")

    xyz_in = [
        nc.dram_tensor(f"xyz{i}", [nb, XYZ_N[i], 3], f32, kind="ExternalInput")
        for i in range(5)
    ]
    f_in = [
        nc.dram_tensor(f"f{i}", [nb, F_C[i], XYZ_N[i]], f32, kind="ExternalInput")
        for i in range(5)
    ]
    out_d = nc.dram_tensor("out", [nb, OUT_C, OUT_N], f32, kind="ExternalOutput")
    # gather tables (stage i gathers from tbl[i]); batch folded into rows so
    # the indirect-DMA offset AP stays the HW-proven 2D [p,1] shape
    # row stride padded to 4096B (2048 fp16) so every gathered row starts
    # page-aligned and a 3840B read never straddles two 4KB pages
    tblpad = [1024, 2048, 2048, 2048]
    tbl = [
        nc.dram_tensor(
            f"tbl{i}", [nb * STAGES[i][3], tblpad[i]], tdt, kind="Internal"
        )
        for i in range(4)
    ]

    AX = mybir.AxisListType.X
    MUL = mybir.AluOpType.mult
    ADD = mybir.AluOpType.add

    with tile.TileContext(nc) as tc, ExitStack() as ctx:
        ident_p = ctx.enter_context(tc.tile_pool(name="ident", bufs=1))
        widep = ctx.enter_context(tc.tile_pool(name="wide", bufs=2))
        abtp = ctx.enter_context(tc.tile_pool(name="abt", bufs=2))
        abtp1 = ctx.enter_context(tc.tile_pool(name="abt1", bufs=2))
        idxp = ctx.enter_context(tc.tile_pool(name="idxp", bufs=2))
        smallp = ctx.enter_context(tc.tile_pool(name="smallp", bufs=4))
        diagp = ctx.enter_context(tc.tile_pool(name="diagp", bufs=3))
        gp = ctx.enter_context(tc.tile_pool(name="gp", bufs=22))
        tcp = ctx.enter_context(tc.tile_pool(name="tcp", bufs=4))
        stagp = ctx.enter_context(tc.tile_pool(name="stagp", bufs=4))
        fblkp = ctx.enter_context(tc.tile_pool(name="fblkp", bufs=2))
        dpool = ctx.enter_context(tc.tile_pool(name="dpool", bufs=2, space="PSUM"))
        mpool = ctx.enter_context(tc.tile_pool(name="mpool", bufs=4, space="PSUM"))

        ident = ident_p.tile([P, P], f32)
        make_identity(nc, ident[:])
        # constant [P, J, 3, P] identity mask used to expand w into diag(w_k)
        idmask = ident_p.tile([P, J, 3, P], tdt)
        for j in range(J):
            for k in range(3):
                nc.vector.tensor_copy(idmask[:, j, k, :], ident[:])

        def pe_transpose_to(dst_ap, src_ap):
            """dst[m, k] = src[k, m] via PE; src [K(partition), M<=128]."""
            k_, m = src_ap.shape[0], src_ap.shape[-1]
            pt = mpool.tile([P, P], f32, tag="po")
            nc.tensor.transpose(pt[:m, :k_], src_ap, ident[:k_, :k_])
            nc.vector.tensor_copy(dst_ap, pt[:m, :k_])

        def pe_transpose_merge(dst_ap, srcs):
            """dst[:, j*k:(j+1)*k] = srcs[j]^T for up to 4 same-shape srcs,
            via one PSUM tile and ONE DVE copy (fewer instructions -> fewer
            semaphore stalls than per-block pe_transpose_to)."""
            k_, m = srcs[0].shape[0], srcs[0].shape[-1]
            pt = mpool.tile([P, 4 * P], f32, tag="po")
            for j, s in enumerate(srcs):
                nc.tensor.transpose(pt[:m, j * k_:(j + 1) * k_], s, ident[:k_, :k_])
            nc.vector.tensor_copy(dst_ap, pt[:m, :len(srcs) * k_])

        def table_chunks(b):
            """Skip-feature table blocks (f^T); yields between blocks."""
            # tbl0 = f4^T entirely: f4 [1024, 16] -> [16, 1024]
            f4w = widep.tile([P, 8, 16], f32, tag="f4w")
            nc.sync.dma_start(f4w[:], f_in[4][b].rearrange("(u p) s -> p u s", p=P))
            t0sb = fblkp.tile([16, 1024], tdt, tag="t0sb")
            for u0 in range(0, 8, 4):
                pe_transpose_merge(
                    t0sb[:, u0 * P:(u0 + 4) * P],
                    [f4w[:, u, :] for u in range(u0, u0 + 4)],
                )
            nc.sync.dma_start(tbl[0][b * 16:(b + 1) * 16, :], t0sb[:])
            yield

            # f3 [512, 64] -> tbl1[:, 0:512]
            f3w = widep.tile([P, 4, 64], f32, tag="f3w")
            nc.sync.dma_start(f3w[:], f_in[3][b].rearrange("(u p) n -> p u n", p=P))
            t1sb = fblkp.tile([64, 512], tdt, tag="t1sb")
            pe_transpose_merge(t1sb[:], [f3w[:, u, :] for u in range(4)])
            nc.sync.dma_start(tbl[1][b * 64:(b + 1) * 64, 0:512], t1sb[:])
            yield

            # f2 [256, 256] -> tbl2[:, 0:256]
            f2w = widep.tile([P, 2, 256], f32, tag="f2w")
            nc.sync.dma_start(f2w[:], f_in[2][b].rearrange("(u p) n -> p u n", p=P))
            for r in range(2):
                t2sb = fblkp.tile([P, 256], tdt, tag="t2sb")
                pe_transpose_merge(
                    t2sb[:], [f2w[:, u, r * P:(r + 1) * P] for u in range(2)])
                nc.sync.dma_start(
                    tbl[2][b * 256 + r * P:b * 256 + (r + 1) * P, 0:256], t2sb[:]
                )
            yield

            # f1 [128, 1024] -> tbl3[:, 0:128]
            f1w = widep.tile([P, 1024], f32, tag="f1w")
            nc.sync.dma_start(f1w[:], f_in[1][b])
            for r0 in range(0, 8, 4):
                t3sb = fblkp.tile([P, 4, P], tdt, tag="t3sb")
                pe_transpose_merge(
                    t3sb[:].rearrange("p j c -> p (j c)"),
                    [f1w[:, r * P:(r + 1) * P] for r in range(r0, r0 + 4)],
                )
                nc.sync.dma_start(
                    tbl[3][
                        b * 1024 + r0 * P:b * 1024 + (r0 + 4) * P, 0:128
                    ].rearrange("(j p) c -> p j c", p=P),
                    t3sb[:],
                )
            yield

            # f0 [64, 4096] -> out rows 0:64 (already channel-major);
            # direct DRAM->DRAM, no SBUF bounce
            for h in range(4):
                nc.sync.dma_start(
                    out_d[b, 0:64, h * 1024:(h + 1) * 1024],
                    f_in[0][b, :, h * 1024:(h + 1) * 1024],
                )
            yield

        def coord_chunks(b, a1t, b2t):
            """Lifted coord operands A'(fine), B'(coarse); yields per level.

            A' = [2x, 2y, 2z, -1, -|p|^2], B' = [x, y, z, |p|^2, 1]
            so that A'.B' = 2ab - |a|^2 - |b|^2 = -d^2
            """
            for i in range(5):
                n = XYZ_N[i]
                t = max(1, n // P)
                pt_ = min(n, P)
                xw = widep.tile([pt_, t, 3], f32, tag="xw")
                nc.sync.dma_start(
                    xw[:], xyz_in[i][b].rearrange("(t p) d -> p t d", p=pt_)
                )
                sq = widep.tile([pt_, t, 3], f32, tag="sq")
                nc.vector.tensor_mul(sq[:], xw[:], xw[:])
                ssum = widep.tile([pt_, t, 1], f32, tag="ssum")
                nc.vector.reduce_sum(ssum[:], sq[:], axis=AX)
                if i != 4:  # used as fine
                    aw = widep.tile([pt_, t, 5], f32, tag="aw")
                    nc.vector.tensor_scalar_mul(aw[:, :, 0:3], xw[:], 2.0)
                    nc.vector.memset(aw[:, :, 3:4], -1.0)
                    nc.vector.tensor_scalar_mul(aw[:, :, 4:5], ssum[:], -1.0)
                    at = (abtp1 if n >= 4096 else abtp).tile(
                        [5, n], f32, tag=f"a1t{i}"
                    )
                    for t0_ in range(0, t, 4):
                        tn = min(4, t - t0_)
                        pe_transpose_merge(
                            at[:, t0_ * pt_:(t0_ + tn) * pt_],
                            [aw[:, tt, :] for tt in range(t0_, t0_ + tn)],
                        )
                    a1t[i] = at
                if i != 0:  # used as coarse
                    bw = widep.tile([pt_, t, 5], f32, tag="bw")
                    nc.vector.tensor_copy(bw[:, :, 0:3], xw[:])
                    nc.vector.tensor_copy(bw[:, :, 3:4], ssum[:])
                    nc.vector.memset(bw[:, :, 4:5], 1.0)
                    bt = (abtp1 if n >= 1024 else abtp).tile(
                        [5, n], f32, tag=f"b2t{i}"
                    )
                    for t0_ in range(0, t, 4):
                        tn = min(4, t - t0_)
                        pe_transpose_merge(
                            bt[:, t0_ * pt_:(t0_ + tn) * pt_],
                            [bw[:, tt, :] for tt in range(t0_, t0_ + tn)],
                        )
                    b2t[i] = bt
                yield

        def make_topk(b, i, at, bt, S, pt_, idxall, valsall):
            def do_topk(t):
                dps = dpool.tile([pt_, S], f32, tag="d")
                for s0 in range(0, S, 512):
                    sw = min(512, S - s0)
                    nc.tensor.matmul(
                        dps[:, s0:s0 + sw],
                        lhsT=at[:, t * pt_:t * pt_ + pt_],
                        rhs=bt[:, s0:s0 + sw],
                        start=True,
                        stop=True,
                    )
                nc.vector.max(out=valsall[:, t, :], in_=dps[:])
                nc.vector.max_index(
                    out=idxall[:, t, :], in_max=valsall[:, t, :], in_values=dps[:]
                )
                if b > 0:
                    # bias to batch-folded table rows; per-tile so gathers
                    # don't wait on the whole stage's topk
                    nc.vector.tensor_scalar_add(
                        idxall[:, t, :], idxall[:, t, :], b * S
                    )
            return do_topk

        def group_weights(valsall, t0, jg, pt_):
            """w = r / sum(r), r = 1/(d + 1e-8), expanded to diag(w_k)."""
            vsl = valsall[:, t0:t0 + jg, :]
            dk = smallp.tile([pt_, jg, 3], f32, tag="dk")
            nc.vector.tensor_scalar(
                out=dk[:], in0=vsl[:, :, 0:3],
                scalar1=-1.0, scalar2=1e-8, op0=MUL, op1=ADD,
            )
            rr = smallp.tile([pt_, jg, 3], f32, tag="rr")
            nc.vector.reciprocal(rr[:], dk[:])
            zz = smallp.tile([pt_, jg, 1], f32, tag="zz")
            nc.vector.reduce_sum(zz[:], rr[:], axis=AX)
            zr = smallp.tile([pt_, jg, 1], f32, tag="zr")
            nc.vector.reciprocal(zr[:], zz[:])
            w3 = smallp.tile([pt_, jg, 3, 1], tdt, tag="w3")
            nc.vector.tensor_tensor(
                out=w3[:, :, :, 0], in0=rr[:],
                in1=zr[:].to_broadcast([pt_, jg, 3]), op=MUL,
            )
            diag = diagp.tile([pt_, jg, 3, pt_], tdt, tag="diag")
            nc.vector.tensor_tensor(
                out=diag[:],
                in0=w3[:].to_broadcast([pt_, jg, 3, pt_]),
                in1=idmask[:pt_, :jg, :, :pt_],
                op=MUL,
            )
            return diag

        def gather_group(i, idxall, t0, jg, pt_, C2):
            # NOTE: the offset AP must stay the HW-proven 2D [p,1] shape; a
            # [p,3] offset AP hard-crashes the exec unit (NRT status 101)
            gs = []
            for tt in range(jg):
                gk = []
                for k in range(3):
                    g = gp.tile([pt_, C2], tdt, tag="g")
                    nc.gpsimd.indirect_dma_start(
                        out=g[:],
                        out_offset=None,
                        in_=tbl[i][:],
                        in_offset=bass.IndirectOffsetOnAxis(
                            ap=idxall[:, t0 + tt, k:k + 1], axis=0
                        ),
                    )
                    gk.append(g)
                gs.append(gk)
            return gs

        def stage_small(b, i, a1t, b2t):
            """Stages 0-2: row-major interp -> next table rows; yields/group."""
            fi, ci, N, S, skipC, C2 = STAGES[i]
            T = max(1, N // P)
            pt_ = min(N, P)
            at, bt = a1t[fi], b2t[ci]

            idxall = idxp.tile([pt_, T, 8], u32, tag=f"idx{i}")
            valsall = smallp.tile([pt_, T, 8], f32, tag=f"vals{i}")
            do_topk = make_topk(b, i, at, bt, S, pt_, idxall, valsall)

            for t in range(min(2 * J, T)):
                do_topk(t)

            for t0 in range(0, T, J):
                jg = min(J, T - t0)
                for t in range(t0 + 2 * J, min(t0 + 3 * J, T)):
                    do_topk(t)
                diag = group_weights(valsall, t0, jg, pt_)
                gs = gather_group(i, idxall, t0, jg, pt_, C2)
                for tt in range(jg):
                    tcst = tcp.tile([pt_, C2], tdt, tag="tc")
                    for c0 in range(0, C2, 512):
                        cw = min(512, C2 - c0)
                        pint = mpool.tile([pt_, 512], f32, tag="po")
                        for k in range(3):
                            nc.tensor.matmul(
                                pint[:, :cw],
                                lhsT=diag[:, tt, k, :],
                                rhs=gs[tt][k][:, c0:c0 + cw],
                                start=(k == 0),
                                stop=(k == 2),
                            )
                        nc.scalar.copy(tcst[:, c0:c0 + cw], pint[:, :cw])
                    r0 = b * STAGES[i + 1][3] + (t0 + tt) * pt_
                    nc.sync.dma_start(
                        tbl[i + 1][r0:r0 + pt_, skipC:skipC + C2], tcst[:]
                    )
                yield

        def stage3(b, a1t, b2t):
            """Final stage: channel-major output rows 64:1984; yields/group.

            topk is software-pipelined two groups ahead of the gathers so PE
            never sits in a stage-wide cdist->topk stall phase.
            """
            fi, ci, N, S, skipC, C2 = STAGES[3]
            T = N // P
            at, bt = a1t[fi], b2t[ci]

            idxall = idxp.tile([P, T, 8], u32, tag="idx3")
            valsall = smallp.tile([P, T, 8], f32, tag="vals3")
            do_topk = make_topk(b, 3, at, bt, S, P, idxall, valsall)

            for t in range(2 * J):
                do_topk(t)
            yield

            for t0 in range(0, T, J):
                jg = min(J, T - t0)
                for t in range(t0 + 2 * J, min(t0 + 3 * J, T)):
                    do_topk(t)
                diag = group_weights(valsall, t0, jg, P)
                gs = gather_group(3, idxall, t0, jg, P, C2)
                for cc in range(C2 // P):
                    pout = mpool.tile([P, 512], f32, tag="po")
                    for tt in range(jg):
                        for k in range(3):
                            nc.tensor.matmul(
                                pout[:, tt * P:(tt + 1) * P],
                                lhsT=gs[tt][k][:, cc * P:(cc + 1) * P],
                                rhs=diag[:, tt, k, :],
                                start=(k == 0),
                                stop=(k == 2),
                            )
                    stg = stagp.tile([P, J * P], f32, tag="stag")
                    cp = nc.scalar.copy if cc % 2 else nc.vector.tensor_copy
                    cp(stg[:, :jg * P], pout[:, :jg * P])
                    nc.sync.dma_start(
                        out_d[
                            b,
                            64 + cc * P:64 + (cc + 1) * P,
                            t0 * P:(t0 + jg) * P,
                        ],
                        stg[:, :jg * P],
                    )
                yield

        def prep(b, a1t, b2t):
            yield from table_chunks(b)
            yield from coord_chunks(b, a1t, b2t)
            for i in range(3):
                yield from stage_small(b, i, a1t, b2t)

        a1t = [{} for _ in range(nb)]
        b2t = [{} for _ in range(nb)]

        for _ in prep(0, a1t[0], b2t[0]):
            pass

        # interleave: stage 3 of elem 0 (DMA-bound) with prep of elem 1
        # (PE-bound), front-loaded 3 prep chunks per stage-3 group so elem 1's
        # stage chain completes early; then bridge elem 1's stage 3 into elem
        # 0's remaining groups so the gather stream never gaps.
        s30 = stage3(0, a1t[0], b2t[0])
        if nb > 1:
            p1 = prep(1, a1t[1], b2t[1])
            while next(p1, _SENT) is not _SENT:
                next(p1, None)
                next(p1, None)
                next(p1, None)
                next(s30, None)
            s31 = stage3(1, a1t[1], b2t[1])
            while True:
                a_done = next(s30, _SENT) is _SENT
                b_done = next(s31, _SENT) is _SENT
                if a_done and b_done:
                    break
        else:
            for _ in s30:
                pass

    nc.compile()
    return nc


_CACHE = {}


def _get_nc():
    if "nc" not in _CACHE:
        _CACHE["nc"] = build_nc(NB)
    return _CACHE["nc"]


def run(inputs, trace=False):
    from concourse.bass_utils import run_bass_kernel_spmd

    if not trace:
        # the axon trace path needs an antenv.axon_hooks shim this image may
        # lack; make plain runs immune to a stray BASS_TRACE in the env
        os.environ["BASS_NEVER_TRACE"] = "1"
    else:
        os.environ.pop("BASS_NEVER_TRACE", None)
    nc = _get_nc()
    in_maps = []
    for c in range(NCORES):
        sl = slice(c * NB, (c + 1) * NB)
        m = {}
        for i in range(5):
            m[f"xyz{i}"] = np.ascontiguousarray(inputs[f"xyz{i}"][sl], dtype=np.float32)
            m[f"f{i}"] = np.ascontiguousarray(inputs[f"f{i}"][sl], dtype=np.float32)
        in_maps.append(m)
    res = run_bass_kernel_spmd(nc, in_maps, core_ids=list(range(NCORES)), trace=trace)
    out = np.concatenate([r["out"] for r in res.results], axis=0)
    return out, res


def kernel(**inputs) -> np.ndarray:
    out, _ = run(inputs)
    return out
